# revision 1
# baseline (speedup 1.0000x reference)
"""Trainium2 Bass kernel for nn_BandProcessor — v2 (pair-granular pipeline).

Structure per 128-token tile (64 tiles/core, batch data-parallel over 8 cores):
  L1: LN1 -> temporal 16-tap causal band (PE band matmul + N=15 spill) ->
      proj (+bias +x residual folded into PSUM) -> x1
  L2: LN2 -> 3-tap neighbor band (N=1 edge columns) -> proj (+bias +x1) -> x2
  FFN: LN3 -> transpose -> W1+gelu -> W2 (+bias +x2) -> out

Key implementation choices:
  - all matmul operands bf16 (f32r only for the x-residual identity matmul of
    the raw f32 input); residual spine x1/x2 kept in bf16, out evac in f32
  - biases via ones-row matmuls, residuals via identity matmuls: PSUM holds
    the full x_{l+1}, one evacuation per pair of tiles
  - LN rstd via DVE bit-trick rsqrt (2 Newton steps), batched over 4 pairs:
    ACT engine never loads an activation table except Gelu (loaded once)
  - LN2/LN3 normalize on the (otherwise idle) GPSIMD engine
  - PSUM pools sized to exactly 8 banks; every PSUM tile is bank-exclusive
"""

import numpy as np
import ml_dtypes

import concourse.bacc as bacc
import concourse.mybir as mybir
from concourse.tile import TileContext
from concourse import bass_utils

B, T, D = 8, 8192, 256
H = 16
DECAY = 0.9
EPS = 1e-5
NT = T // 128           # 64 token tiles per core
NP = NT // 2            # 32 pairs
SBP = 8                 # pairs per superblock (FFN batching) -> 2048 tokens
NSB = NP // SBP         # 4 superblocks
RB = 4                  # pairs per rsqrt batch (8 tiles)

F32 = mybir.dt.float32
F32R = mybir.dt.float32r
BF16 = mybir.dt.bfloat16
I32 = mybir.dt.int32

AF = mybir.ActivationFunctionType
ALU = mybir.AluOpType
MAGIC = 0x5F3759DF


# ---------------------------------------------------------------- host prep

def _host_consts(inp, gelu_ok=True):
    """Fold LN gains + value/out projections into single matrices (f64)."""
    g1, b1_ = inp["n1_g"].astype(np.float64), inp["n1_b"].astype(np.float64)
    g2, b2_ = inp["n2_g"].astype(np.float64), inp["n2_b"].astype(np.float64)
    g3, b3_ = inp["n3_g"].astype(np.float64), inp["n3_b"].astype(np.float64)
    t_Wv, t_bv = inp["t_Wv"].astype(np.float64), inp["t_bv"].astype(np.float64)
    t_Wo, t_bo = inp["t_Wo"].astype(np.float64), inp["t_bo"].astype(np.float64)
    a_Wv, a_bv = inp["a_Wv"].astype(np.float64), inp["a_bv"].astype(np.float64)
    a_Wo, a_bo = inp["a_Wo"].astype(np.float64), inp["a_bo"].astype(np.float64)
    f_W1, f_b1 = inp["f_W1"].astype(np.float64), inp["f_b1"].astype(np.float64)
    f_W2, f_b2 = inp["f_W2"].astype(np.float64), inp["f_b2"].astype(np.float64)

    WtWo = t_Wv @ t_Wo
    WaWo = a_Wv @ a_Wo
    Wt_eff = g1[:, None] * WtWo                                # [D, D]
    bt_eff = b1_ @ WtWo + t_bv @ t_Wo + t_bo
    Wa_eff = g2[:, None] * WaWo
    ba_eff = b2_ @ WaWo + a_bv @ a_Wo + a_bo
    W1_eff = g3[:, None] * f_W1                                # [D, 2D]
    b1_eff = b3_ @ f_W1 + f_b1                                 # [2D]
    W2 = f_W2
    b2v = f_b2

    # temporal weights: reference tw[j] applies to h_pad[j:j+T];
    # lag d = H-1-j  ->  w_lag[d] = tw[H-1-d]
    tw = DECAY ** np.arange(H, dtype=np.float64)
    tw = tw / tw.sum()
    w_lag = tw[::-1].copy()

    band1c = np.zeros((128, 128), np.float64)
    for ti in range(128):
        for to in range(ti, min(128, ti + H)):
            band1c[ti, to] = w_lag[to - ti]
    # spill into next tile: cols 0..14 get lags 1..15 from prev tile's
    # partitions 113..127
    band1p15 = np.zeros((128, 15), np.float64)
    for p in range(113, 128):
        for to in range(0, p - 112):
            band1p15[p, to] = w_lag[to + 128 - p]
    band2c = np.zeros((128, 128), np.float64)
    for ti in range(128):
        for to in range(max(0, ti - 1), min(128, ti + 2)):
            band2c[ti, to] = 1.0 / 3.0
    ep_col = np.zeros((128, 1), np.float64); ep_col[127, 0] = 1.0 / 3.0
    ep0_col = np.zeros((128, 1), np.float64); ep0_col[0, 0] = 1.0 / 3.0
    en_col = np.zeros((128, 1), np.float64); en_col[0, 0] = 1.0 / 3.0
    en63_col = np.zeros((128, 1), np.float64); en63_col[127, 0] = 1.0 / 3.0

    bf = lambda a: np.ascontiguousarray(a).astype(ml_dtypes.bfloat16)
    f32 = lambda a: np.ascontiguousarray(a).astype(np.float32)

    # first-tile correction for the temporal zero-pad of the LN bias term
    c_t = np.cumsum(w_lag)[:H - 1]
    corr = f32((c_t - 1.0)[:, None] * (b1_ @ WtWo)[None, :])

    consts = {
        "wt": bf(np.stack([Wt_eff[0:128], Wt_eff[128:256]])),       # [2,128,256]
        "wa": bf(np.stack([Wa_eff[0:128], Wa_eff[128:256]])),
        "w1": bf(np.stack([W1_eff[0:128], W1_eff[128:256]])),       # [2,128,512]
        "w2": bf(np.stack([W2[k * 128:(k + 1) * 128] for k in range(4)])),
        "band1c": bf(band1c), "band1p15": bf(band1p15),
        "band2c": bf(band2c),
        "ecols": bf(np.concatenate([ep_col, ep0_col, en_col, en63_col], axis=1)),
        "ones_r": bf(np.ones((1, 128))),
        # biases duplicated for pair-wide N=512 matmuls
        "browp": bf(np.stack([np.tile(bt_eff, 2), np.tile(ba_eff, 2),
                              np.tile(b2v, 2)])),                   # [3,512]
        "b1col": f32(b1_eff.reshape(4, 128).T),                     # [128,4]
        "identb": bf(np.eye(128)),
        "identf": f32(np.eye(128)),
    }
    need_corr = bool(np.abs(corr).max() > 0)
    return consts, corr, need_corr


# ---------------------------------------------------------------- bass build

def build_nc(repeat=1, need_corr=False, gelu=True):
    nc = bacc.Bacc("TRN2", target_bir_lowering=False, debug=False, num_devices=8)
    GELU = AF.Gelu if gelu else AF.Identity

    x_d = nc.dram_tensor("x", (T, D), F32R, kind="ExternalInput")
    out_d = nc.dram_tensor("out", (T, D), F32, kind="ExternalOutput")
    wt_d = nc.dram_tensor("wt", (2, 128, 256), BF16, kind="ExternalInput")
    wa_d = nc.dram_tensor("wa", (2, 128, 256), BF16, kind="ExternalInput")
    w1_d = nc.dram_tensor("w1", (2, 128, 512), BF16, kind="ExternalInput")
    w2_d = nc.dram_tensor("w2", (4, 128, 256), BF16, kind="ExternalInput")
    b1c_d = nc.dram_tensor("band1c", (128, 128), BF16, kind="ExternalInput")
    b1p_d = nc.dram_tensor("band1p15", (128, 15), BF16, kind="ExternalInput")
    b2c_d = nc.dram_tensor("band2c", (128, 128), BF16, kind="ExternalInput")
    ec_d = nc.dram_tensor("ecols", (128, 4), BF16, kind="ExternalInput")
    ones_d = nc.dram_tensor("ones_r", (1, 128), BF16, kind="ExternalInput")
    browp_d = nc.dram_tensor("browp", (3, 512), BF16, kind="ExternalInput")
    b1col_d = nc.dram_tensor("b1col", (128, 4), F32, kind="ExternalInput")
    idb_d = nc.dram_tensor("identb", (128, 128), BF16, kind="ExternalInput")
    idf_d = nc.dram_tensor("identf", (128, 128), F32R, kind="ExternalInput")
    corr_d = nc.dram_tensor("corr", (15, 256), F32, kind="ExternalInput") if need_corr else None

    with TileContext(nc) as tc:
        import contextlib
        ctx = contextlib.ExitStack()
        with ctx:
            consts = ctx.enter_context(tc.tile_pool(name="consts", bufs=1))
            xpool = ctx.enter_context(tc.tile_pool(name="xpool", bufs=9))
            xn1p = ctx.enter_context(tc.tile_pool(name="xn1p", bufs=8))
            a1p = ctx.enter_context(tc.tile_pool(name="a1p", bufs=3))
            x1p = ctx.enter_context(tc.tile_pool(name="x1p", bufs=10))
            xn2p = ctx.enter_context(tc.tile_pool(name="xn2p", bufs=10))
            a2p = ctx.enter_context(tc.tile_pool(name="a2p", bufs=3))
            x2p = ctx.enter_context(tc.tile_pool(name="x2p", bufs=22))
            xn3p = ctx.enter_context(tc.tile_pool(name="xn3p", bufs=8))
            mvp = ctx.enter_context(tc.tile_pool(name="mvp", bufs=3))
            rsp = ctx.enter_context(tc.tile_pool(name="rsp", bufs=3))
            bigp = ctx.enter_context(tc.tile_pool(name="bigp", bufs=2))
            gelup = ctx.enter_context(tc.tile_pool(name="gelup", bufs=2))
            outp = ctx.enter_context(tc.tile_pool(name="outp", bufs=3))
            smalls = ctx.enter_context(tc.tile_pool(name="smalls", bufs=8))
            # PSUM: 8 banks total, every tile 1 full bank; per-tag bufs:
            # agg1 1 + agg2 1 + x3t 1 + att1 1 + att2 1 + att3 1 + gps 2 = 8
            pps = ctx.enter_context(tc.tile_pool(name="pps", bufs=1, space="PSUM"))

            # ---- load constants once
            wt_sb = consts.tile([128, 2, 256], BF16)
            wa_sb = consts.tile([128, 2, 256], BF16)
            w1_sb = consts.tile([128, 2, 512], BF16)
            w2_sb = consts.tile([128, 4, 256], BF16)
            for k in range(2):
                nc.sync.dma_start(out=wt_sb[:, k, :], in_=wt_d[k, :, :])
                nc.sync.dma_start(out=wa_sb[:, k, :], in_=wa_d[k, :, :])
                nc.sync.dma_start(out=w1_sb[:, k, :], in_=w1_d[k, :, :])
            for k in range(4):
                nc.sync.dma_start(out=w2_sb[:, k, :], in_=w2_d[k, :, :])
            band1c = consts.tile([128, 128], BF16, tag="b1c")
            nc.sync.dma_start(out=band1c, in_=b1c_d[:, :])
            band1p = consts.tile([128, 15], BF16, tag="b1p")
            nc.sync.dma_start(out=band1p, in_=b1p_d[:, :])
            band2c = consts.tile([128, 128], BF16, tag="b2c")
            nc.sync.dma_start(out=band2c, in_=b2c_d[:, :])
            ecols = consts.tile([128, 4], BF16, tag="ec")
            nc.sync.dma_start(out=ecols, in_=ec_d[:, :])
            ones_sb = consts.tile([1, 128], BF16, tag="ones")
            nc.sync.dma_start(out=ones_sb, in_=ones_d[:, :])
            browp_sb = consts.tile([1, 3, 512], BF16, tag="browp")
            nc.sync.dma_start(out=browp_sb, in_=browp_d[:, :])
            b1_sb = consts.tile([128, 4], F32, tag="b1c2")
            nc.sync.dma_start(out=b1_sb, in_=b1col_d[:, :])
            idb_sb = consts.tile([128, 128], BF16, tag="idb")
            nc.sync.dma_start(out=idb_sb, in_=idb_d[:, :])
            idf_sb = consts.tile([128, 128], F32R, tag="idf")
            nc.sync.dma_start(out=idf_sb, in_=idf_d[:, :])
            magic_sb = consts.tile([128, 2 * RB], I32, tag="magic")
            nc.vector.memset(magic_sb, MAGIC)
            corr_sb = None
            if need_corr:
                corr_sb = consts.tile([15, 256], F32, tag="corr")
                nc.sync.dma_start(out=corr_sb, in_=corr_d[:, :])

            st = {}

            # ---------------- helpers

            def stats(src_ap, mvb, slot):
                """bn_stats/aggr for one tile -> mvb[:, slot, 0:2]."""
                s6 = smalls.tile([128, 6], F32, tag="bn6")
                nc.vector.bn_stats(s6, src_ap)
                nc.vector.bn_aggr(mvb[:, slot, :], s6)

            def rsqrt_batch(mvb, rstd):
                """rstd[:, j] = 1/sqrt(var_j) for 2*RB tiles; DVE only."""
                n = 2 * RB
                var = smalls.tile([128, n], F32, tag="rs_v")
                nc.vector.tensor_copy(out=var, in_=mvb[:, :, 1:2])
                t1 = smalls.tile([128, n], I32, tag="rs_t1")
                nc.vector.tensor_scalar(out=t1, in0=var.bitcast(I32), scalar1=1,
                                        scalar2=None, op0=ALU.logical_shift_right)
                y0 = smalls.tile([128, n], I32, tag="rs_y0")
                nc.vector.tensor_tensor(out=y0, in0=magic_sb, in1=t1, op=ALU.subtract)
                y0f = y0.bitcast(F32)
                a = smalls.tile([128, n], F32, tag="rs_a")
                nc.vector.tensor_tensor(out=a, in0=y0f, in1=y0f, op=ALU.mult)
                b = smalls.tile([128, n], F32, tag="rs_b")
                nc.vector.tensor_tensor(out=b, in0=var, in1=a, op=ALU.mult)
                c = smalls.tile([128, n], F32, tag="rs_c")
                nc.vector.tensor_scalar(out=c, in0=b, scalar1=-0.5, scalar2=1.5,
                                        op0=ALU.mult, op1=ALU.add)
                y1 = smalls.tile([128, n], F32, tag="rs_y1")
                nc.vector.tensor_tensor(out=y1, in0=y0f, in1=c, op=ALU.mult)
                nc.vector.tensor_tensor(out=a, in0=y1, in1=y1, op=ALU.mult)
                nc.vector.tensor_tensor(out=b, in0=var, in1=a, op=ALU.mult)
                nc.vector.tensor_scalar(out=c, in0=b, scalar1=-0.5, scalar2=1.5,
                                        op0=ALU.mult, op1=ALU.add)
                nc.vector.tensor_tensor(out=rstd, in0=y1, in1=c, op=ALU.mult)

            # ---------------- stage functions (pair-granular) ----------------

            def sA(p):
                """DMA x pair p; LN1 stats."""
                xp = xpool.tile([128, 2, 256], F32R, tag="x")
                lo = p * 256
                nc.sync.dma_start(
                    out=xp, in_=x_d[lo:lo + 256, :].rearrange("(a p) d -> p a d", a=2))
                st[("x", p)] = xp
                if p % RB == 0:
                    st[("mv1", p // RB)] = mvp.tile([128, 2 * RB, 2], F32, tag="mv1", name="mv1")
                mvb = st[("mv1", p // RB)]
                for t in range(2):
                    stats(xp[:, t, :].bitcast(F32), mvb, (p % RB) * 2 + t)
                if p % RB == RB - 1:
                    r = rsp.tile([128, 2 * RB], F32, tag="rstd1")
                    rsqrt_batch(mvb, r)
                    st[("rstd1", p // RB)] = r

            def sA2(p):
                """LN1 normalize -> xn1 bf16 (DVE; x is f32)."""
                xp = st[("x", p)]
                mvb = st[("mv1", p // RB)]
                r = st[("rstd1", p // RB)]
                xn = xn1p.tile([128, 2, 256], BF16, tag="xn1")
                for t in range(2):
                    j = (p % RB) * 2 + t
                    nc.vector.tensor_scalar(out=xn[:, t, :], in0=xp[:, t, :].bitcast(F32),
                                            scalar1=mvb[:, j, 0:1], scalar2=r[:, j:j + 1],
                                            op0=ALU.subtract, op1=ALU.mult)
                st[("xn1", p)] = xn

            def sB(p):
                """Temporal band matmuls for pair p -> agg1 PSUM; evac -> a1sb."""
                xn = st[("xn1", p)]
                xnm = st.get(("xn1", p - 1))
                agg = pps.tile([128, 2, 2, 128], F32, tag="aggT", bufs=2, name="agg1")
                for t in range(2):
                    g = 2 * p + t
                    prev = xn[:, 0, :] if t == 1 else (xnm[:, 1, :] if xnm is not None else None)
                    for h in range(2):
                        hs = slice(h * 128, (h + 1) * 128)
                        nc.tensor.matmul(agg[:, t, h, :], xn[:, t, hs], band1c,
                                         start=True, stop=(g == 0))
                        if g > 0:
                            nc.tensor.matmul(agg[:, t, h, 0:15], prev[:, hs], band1p,
                                             start=False, stop=True)
                a1 = a1p.tile([128, 2, 2, 128], BF16, tag="a1sb")
                nc.vector.tensor_copy(out=a1, in_=agg)
                st[("a1sb", p)] = a1
                st.pop(("xn1", p - 1), None)

            def sC(p):
                """proj1 + bias + x residual -> A1 PSUM; evac -> x1 pair bf16."""
                a1 = st.pop(("a1sb", p))
                att = pps.tile([128, 2, 256], F32, tag="att1", bufs=2, name="att1")
                for t in range(2):
                    for h in range(2):
                        nc.tensor.matmul(att[:, t, :], a1[:, t, h, :], wt_sb[:, h, :],
                                         start=(t == 0 and h == 0), stop=False)
                attf = att.rearrange("p a d -> p (a d)")
                nc.tensor.matmul(attf, ones_sb, browp_sb[:, 0, :], start=False, stop=False)
                xp = st.pop(("x", p))
                nc.tensor.matmul(attf, idf_sb,
                                 xp.rearrange("p a d -> p (a d)"),
                                 start=False, stop=True)
                x1 = x1p.tile([128, 2, 256], BF16, tag="x1")
                nc.scalar.activation(x1, att, AF.Copy)
                if need_corr and p == 0:
                    nc.vector.tensor_tensor(out=x1[0:15, 0, :], in0=x1[0:15, 0, :],
                                            in1=corr_sb, op=ALU.add)
                st[("x1", p)] = x1

            def sD(p):
                """LN2 stats on x1."""
                x1 = st[("x1", p)]
                if p % RB == 0:
                    st[("mv2", p // RB)] = mvp.tile([128, 2 * RB, 2], F32, tag="mv2", name="mv2")
                mvb = st[("mv2", p // RB)]
                for t in range(2):
                    stats(x1[:, t, :], mvb, (p % RB) * 2 + t)
                if p % RB == RB - 1:
                    r = rsp.tile([128, 2 * RB], F32, tag="rstd2")
                    rsqrt_batch(mvb, r)
                    st[("rstd2", p // RB)] = r

            def sD2(p):
                """LN2 normalize on GPSIMD -> xn2 bf16."""
                x1 = st[("x1", p)]
                mvb = st[("mv2", p // RB)]
                r = st[("rstd2", p // RB)]
                xn = xn2p.tile([128, 2, 256], BF16, tag="xn2")
                for t in range(2):
                    j = (p % RB) * 2 + t
                    nc.vector.tensor_scalar(out=xn[:, t, :], in0=x1[:, t, :],
                                            scalar1=mvb[:, j, 0:1], scalar2=r[:, j:j + 1],
                                            op0=ALU.subtract, op1=ALU.mult)
                st[("xn2", p)] = xn

            def sE(p):
                """Neighbor band for pair p (needs xn2 of pairs p-1, p, p+1)."""
                xn = st[("xn2", p)]
                xnm = st.get(("xn2", p - 1))
                xnp = st.get(("xn2", p + 1))
                agg = pps.tile([128, 2, 2, 128], F32, tag="aggT", bufs=2, name="agg2")
                for t in range(2):
                    g = 2 * p + t
                    prev = xn[:, 0, :] if t == 1 else (xnm[:, 1, :] if xnm is not None else None)
                    nxt = xn[:, 1, :] if t == 0 else (xnp[:, 0, :] if xnp is not None else None)
                    for h in range(2):
                        hs = slice(h * 128, (h + 1) * 128)
                        nc.tensor.matmul(agg[:, t, h, :], xn[:, t, hs], band2c,
                                         start=True, stop=False)
                        if g > 0:
                            nc.tensor.matmul(agg[:, t, h, 0:1], prev[:, hs],
                                             ecols[:, 0:1], start=False, stop=False)
                        else:
                            nc.tensor.matmul(agg[:, t, h, 0:1], xn[:, t, hs],
                                             ecols[:, 1:2], start=False, stop=False)
                        if g < NT - 1:
                            nc.tensor.matmul(agg[:, t, h, 127:128], nxt[:, hs],
                                             ecols[:, 2:3], start=False, stop=True)
                        else:
                            nc.tensor.matmul(agg[:, t, h, 127:128], xn[:, t, hs],
                                             ecols[:, 3:4], start=False, stop=True)
                a2 = a2p.tile([128, 2, 2, 128], BF16, tag="a2sb")
                nc.scalar.activation(a2, agg, AF.Copy)
                st[("a2sb", p)] = a2
                if p - 2 >= 0:
                    st.pop(("xn2", p - 2), None)

            def sF(p):
                """proj2 + bias + x1 residual -> A2; evac -> x2 pair bf16."""
                a2 = st.pop(("a2sb", p))
                att = pps.tile([128, 2, 256], F32, tag="att2", bufs=2, name="att2")
                for t in range(2):
                    for h in range(2):
                        nc.tensor.matmul(att[:, t, :], a2[:, t, h, :], wa_sb[:, h, :],
                                         start=(t == 0 and h == 0), stop=False)
                attf = att.rearrange("p a d -> p (a d)")
                nc.tensor.matmul(attf, ones_sb, browp_sb[:, 1, :], start=False, stop=False)
                x1 = st.pop(("x1", p))
                nc.tensor.matmul(attf, idb_sb, x1.rearrange("p a d -> p (a d)"),
                                 start=False, stop=True)
                x2 = x2p.tile([128, 2, 256], BF16, tag="x2")
                nc.scalar.activation(x2, att, AF.Copy)
                st[("x2", p)] = x2

            def sG(p):
                """LN3 stats."""
                x2 = st[("x2", p)]
                if p % RB == 0:
                    st[("mv3", p // RB)] = mvp.tile([128, 2 * RB, 2], F32, tag="mv3", name="mv3")
                mvb = st[("mv3", p // RB)]
                for t in range(2):
                    stats(x2[:, t, :], mvb, (p % RB) * 2 + t)
                if p % RB == RB - 1:
                    r = rsp.tile([128, 2 * RB], F32, tag="rstd3")
                    rsqrt_batch(mvb, r)
                    st[("rstd3", p // RB)] = r

            def sG2(p):
                """LN3 normalize on GPSIMD -> xn3 bf16."""
                x2 = st[("x2", p)]
                mvb = st[("mv3", p // RB)]
                r = st[("rstd3", p // RB)]
                xn = xn3p.tile([128, 2, 256], BF16, tag="xn3")
                for t in range(2):
                    j = (p % RB) * 2 + t
                    nc.vector.tensor_scalar(out=xn[:, t, :], in0=x2[:, t, :],
                                            scalar1=mvb[:, j, 0:1], scalar2=r[:, j:j + 1],
                                            op0=ALU.subtract, op1=ALU.mult)
                st[("xn3", p)] = xn

            def sH(p):
                """Transpose xn3 pair -> PSUM [128,(h,t),128]; evac -> xn3T."""
                xn = st.pop(("xn3", p))
                tp = pps.tile([128, 2, 2, 128], BF16, tag="aggT", bufs=2, name="x3t")
                for t in range(2):
                    for h in range(2):
                        nc.tensor.transpose(tp[:, h, t, :], xn[:, t, h * 128:(h + 1) * 128],
                                            idb_sb)
                sbn = p // SBP
                buf = st[("xn3T", sbn)]
                lo = (p % SBP) * 256
                dst = buf[:, :, lo:lo + 256]
                src = tp.rearrange("p h t k -> p h (t k)")
                if p % 2 == 0:
                    nc.scalar.activation(dst, src, AF.Copy)
                else:
                    nc.vector.tensor_copy(out=dst, in_=src)

            def ffn1_unit(sbn, q, m):
                """One FFN1 (q,m) unit: 2 matmuls + gelu."""
                xbuf = st[("xn3T", sbn)]
                if ("gbuf", sbn) not in st:
                    st[("gbuf", sbn)] = gelup.tile([128, 4, SBP * 256], BF16,
                                                   tag="gbuf", name="gbuf")
                gbuf = st[("gbuf", sbn)]
                qs = slice(q * 512, (q + 1) * 512)
                gps = pps.tile([128, 512], F32, tag="gps", bufs=1, name="gps")
                ms = slice(m * 128, (m + 1) * 128)
                nc.tensor.matmul(gps, w1_sb[:, 0, ms], xbuf[:, 0, qs],
                                 start=True, stop=False)
                nc.tensor.matmul(gps, w1_sb[:, 1, ms], xbuf[:, 1, qs],
                                 start=False, stop=True)
                nc.scalar.activation(gbuf[:, m, qs], gps, GELU,
                                     bias=b1_sb[:, m:m + 1])

            def sJ(p):
                """FFN2 + bias + x2 residual -> A3; evac out pair f32; DMA."""
                sbn = p // SBP
                gbuf = st[("gbuf", sbn)]
                att = pps.tile([128, 2, 256], F32, tag="att3", bufs=1, name="att3")
                for t in range(2):
                    cs = slice((p % SBP) * 256 + t * 128, (p % SBP) * 256 + (t + 1) * 128)
                    for k in range(4):
                        nc.tensor.matmul(att[:, t, :], gbuf[:, k, cs], w2_sb[:, k, :],
                                         start=(t == 0 and k == 0), stop=False)
                attf = att.rearrange("p a d -> p (a d)")
                nc.tensor.matmul(attf, ones_sb, browp_sb[:, 2, :], start=False, stop=False)
                x2 = st.pop(("x2", p))
                nc.tensor.matmul(attf, idb_sb, x2.rearrange("p a d -> p (a d)"),
                                 start=False, stop=True)
                ot = outp.tile([128, 2, 256], F32, tag="out")
                nc.scalar.activation(ot, att, AF.Copy)
                lo = p * 256
                nc.sync.dma_start(
                    out=out_d[lo:lo + 256, :].rearrange("(a p) d -> p a d", a=2), in_=ot)
                if p % SBP == SBP - 1:
                    st.pop(("gbuf", sbn), None)

            # ---------------- emission: software-pipelined over pairs

            def body():
                st.clear()
                for s_ in range(NSB):
                    st[("xn3T", s_)] = bigp.tile([128, 2, SBP * 256], BF16, tag="xn3T", name="xn3T")
                stages = [(sA, 0), (sA2, 3), (sB, 4), (sC, 5), (sD, 6), (sD2, 9),
                          (sE, 10), (sF, 12), (sG, 13), (sG2, 16), (sH, 17)]
                import collections as _c
                jq = _c.deque()
                uq = _c.deque()

                def emit_unit():
                    if uq:
                        sbn, q, m = uq.popleft()
                        ffn1_unit(sbn, q, m)

                for s_ in range(NP + 23 + 2 * SBP):
                    for fn, d_ in stages:
                        i = s_ - d_
                        if 0 <= i < NP:
                            fn(i)
                        if fn in (sB, sE, sH):
                            emit_unit()
                    ph = s_ - 17
                    if 0 <= ph < NP and ph % SBP == SBP - 1:
                        sbn = ph // SBP
                        uq.extend((sbn, q, m) for q in range(4) for m in range(4))
                        jq.extend(range(ph - SBP + 1, ph + 1))
                        emit_unit()
                        emit_unit()
                    else:
                        emit_unit()
                    if jq and s_ >= SBP * (jq[0] // SBP) + 25 + (jq[0] % SBP) // 2:
                        sJ(jq.popleft())
                    emit_unit()
                while uq:
                    emit_unit()
                while jq:
                    sJ(jq.popleft())

            if repeat > 1:
                with tc.For_i(0, repeat, 1):
                    body()
            else:
                body()

    nc.compile()
    return nc


# ---------------------------------------------------------------- entry

def _run(inputs, repeat=1, n_calls=1, gelu=True):
    import time
    consts, corr, need_corr = _host_consts(inputs)
    nc = build_nc(repeat=repeat, need_corr=need_corr, gelu=gelu)
    x = np.asarray(inputs["x"], np.float32)
    in_maps = []
    for b in range(B):
        m = {"x": np.ascontiguousarray(x[b])}
        for k, v in consts.items():
            m[k] = v
        if need_corr:
            m["corr"] = corr
        in_maps.append(m)
    times = []
    res = None
    for _ in range(n_calls):
        t0 = time.time()
        res = bass_utils.run_bass_kernel_spmd(nc, in_maps, core_ids=list(range(B)))
        times.append(time.time() - t0)
    out = np.stack([res.results[b]["out"] for b in range(B)]).astype(np.float32)
    return out, times


def kernel(**inputs) -> np.ndarray:
    try:
        out, _ = _run(inputs, repeat=1, n_calls=1)
    except Exception:
        # transient device wedges have been observed; one retry
        out, _ = _run(inputs, repeat=1, n_calls=1)
    return out

